# revision 10
# baseline (speedup 1.0000x reference)
"""CrossNet (DCN cross layers) forward on 8 Trainium2 NeuronCores.

Math: xl_{t+1} = x0 * (xl_t . w_t) + b_t + xl_t  stays in the affine span of
x0: xl_t = a_t * x0 + c_t with c_t = sum_{j<t} b_j a constant vector and a_t a
per-row scalar.  With u_t = x0 . w_t and g_t = c_t . w_t (weight-only consts):

    a_{t+1} = a_t * (1 + u_t) + g_t ,  a_0 = 1
    out     = a_L * x0 + sum_t b_t

So the whole network is 3 independent per-row dot products (u0,u1,u2), a tiny
scalar recurrence, and one scale-and-add -- one read of x, one write of out.

Sharding: data-parallel on batch, 2048 rows per core, weights replicated.

Engine split per [128,1024] row tile (DMA roofline ~2.9us/tile/core):
  DVE    u0,u1 via tensor_tensor_reduce (fp32 1x, no gpsimd port contention)
  GPSIMD u2 via scalar_tensor_tensor with accum_out
  ACT    recurrence (3 tiny activations), diag(a3) build, PSUM->SBUF copy
  PE     out_psum = diag(a3) @ x0 + ones x dsum   (diagonal-matmul row scale)
  DMA    HWDGE loads/stores, 512KB each
"""

import os

import numpy as np

import concourse.mybir as mybir
from concourse.bacc import Bacc
from concourse.bass import Bass
from concourse.bass_utils import run_bass_kernel_spmd
from concourse.masks import make_identity
from concourse.tile import TileContext

B, D, L = 16384, 1024, 3
N_CORES = 8
RPC = B // N_CORES  # rows per core: 2048
P = 128
N_TILES = RPC // P  # 16
F32 = mybir.dt.float32

# u2 engine: "gpsimd" offloads the third dot product to the Q7 cores,
# "vector" keeps all three on DVE.
U2_ENGINE = os.environ.get("CROSSNET_U2", "gpsimd")
# Benchmark-only: repeat the whole body N times inside one program so
# per-iteration HW time can be extracted as a slope (axon has no NTFF).
REPEAT = int(os.environ.get("CROSSNET_REPEAT", "1"))

_CACHE: dict[str, Bass] = {}


def _build() -> Bass:
    # Bacc (not plain Bass): its compile() runs move_matmul_waits_to_ldweights
    # + generate_event_semaphores, which walrus needs -- matmul/LDW can carry
    # at most one sync wait.
    nc = Bacc("TRN2", target_bir_lowering=False, debug=False, num_devices=N_CORES)
    x = nc.dram_tensor("x", [RPC, D], F32, kind="ExternalInput")
    # aux rows: 0..2 = w0,w1,w2 ; 3 = dsum ; 4 = [g1, g2, 0...]
    aux = nc.dram_tensor("aux", [5, D], F32, kind="ExternalInput")
    out = nc.dram_tensor("out", [RPC, D], F32, kind="ExternalOutput")

    mult = mybir.AluOpType.mult
    add = mybir.AluOpType.add
    bypass = mybir.AluOpType.bypass
    Copy = mybir.ActivationFunctionType.Copy
    Ident = mybir.ActivationFunctionType.Identity

    with TileContext(nc) as tc:
        with (
            tc.tile_pool(name="consts", bufs=1) as consts,
            tc.tile_pool(name="xp", bufs=6) as xp,
            tc.tile_pool(name="op", bufs=4) as op,
            tc.tile_pool(name="small", bufs=6) as small,
            tc.tile_pool(name="diagp", bufs=3) as diagp,
            tc.tile_pool(name="psum", bufs=2, space="PSUM") as psum_pool,
            tc.tile_pool(name="psum_bc", bufs=2, space="PSUM") as psum_bc,
        ):
            # ---- one-time constants ----
            # Each aux row lands in its own [1, D] tile at partition 0 so it
            # can be a matmul rhs alongside lhsT tiles based at partition 0.
            w_row = []
            for t in range(3):
                r = consts.tile([1, D], F32, tag=f"w_row{t}")
                nc.sync.dma_start(out=r, in_=aux[t : t + 1, :])
                w_row.append(r)
            dsum_row = consts.tile([1, D], F32, tag="dsum_row")
            nc.sync.dma_start(out=dsum_row, in_=aux[3:4, :])
            g_row = consts.tile([1, 2], F32, tag="g_row")
            nc.sync.dma_start(out=g_row, in_=aux[4:5, 0:2])

            ones_col = consts.tile([1, P], F32, tag="ones_col")
            nc.vector.memset(ones_col, 1.0)
            ident = consts.tile([P, P], F32, tag="ident")
            make_identity(nc, ident)

            # Broadcast w_t and [g1,g2] across partitions via PE outer
            # product: ones[1,P]^T @ row[1,N] -> [P, N] in PSUM.
            wb = []
            for t in range(3):
                wt = consts.tile([P, D], F32, tag=f"wb{t}")
                for h in range(2):
                    sl = slice(512 * h, 512 * (h + 1))
                    ps = psum_bc.tile([P, 512], F32, tag="bc")
                    nc.tensor.matmul(ps, ones_col, w_row[t][:, sl], start=True, stop=True)
                    nc.scalar.copy(wt[:, sl], ps)
                wb.append(wt)
            gb = consts.tile([P, 2], F32, tag="gb")
            ps = psum_bc.tile([P, 2], F32, tag="bc_g")
            nc.tensor.matmul(ps, ones_col, g_row[:, 0:2], start=True, stop=True)
            nc.scalar.copy(gb, ps)

            # ---- steady-state row tiles ----
            for i in range(N_TILES * REPEAT):
                i = i % N_TILES
                rows = slice(i * P, (i + 1) * P)
                xt = xp.tile([P, D], F32, tag="x")
                nc.sync.dma_start(out=xt, in_=x[rows, :])

                # Per-row dot products u_t = x . w_t.  tensor_tensor_reduce
                # is an Anthropic-custom DVE op whose ucode table doesn't
                # reach the device on this runtime path (DVE faults), so use
                # the native TensorScalarPtr form: out = (x*1) * w_t with a
                # free-dim accumulator.
                u = small.tile([P, 4], F32, tag="u")
                scr = small.tile([P, D], F32, tag="scr")
                for t in range(2):
                    nc.vector.scalar_tensor_tensor(
                        out=scr,
                        in0=xt,
                        scalar=1.0,
                        in1=wb[t],
                        op0=mult,
                        op1=mult,
                        accum_out=u[:, t : t + 1],
                    )
                if U2_ENGINE == "gpsimd":
                    # Pool engine has no TensorScalarPtr opcode and its
                    # tensor_reduce is partition-axis only: multiply into a
                    # scratch tile on GPSIMD, reduce on ACT via accum_out.
                    scr2 = xp.tile([P, D], F32, tag="scr2")
                    nc.gpsimd.tensor_tensor(scr2, xt, wb[2], op=mult)
                    dummy2 = small.tile([P, 1], F32, tag="dummy2")
                    nc.scalar.activation(
                        dummy2.broadcast_to((P, D)),
                        scr2,
                        Copy,
                        accum_out=u[:, 2:3],
                    )
                else:
                    nc.vector.scalar_tensor_tensor(
                        out=scr,
                        in0=xt,
                        scalar=1.0,
                        in1=wb[2],
                        op0=mult,
                        op1=mult,
                        accum_out=u[:, 2:3],
                    )

                # recurrence: v = 1+u ; a2 = v0*v1+g1 ; a3 = a2*v2+g2
                v = small.tile([P, 3], F32, tag="v")
                nc.scalar.activation(v, u[:, 0:3], Ident, bias=1.0)
                a2 = small.tile([P, 1], F32, tag="a2")
                nc.scalar.activation(
                    a2, v[:, 0:1], Ident, scale=v[:, 1:2], bias=gb[:, 0:1]
                )
                a3 = small.tile([P, 1], F32, tag="a3")
                nc.scalar.activation(
                    a3, a2, Ident, scale=v[:, 2:3], bias=gb[:, 1:2]
                )
                diag = diagp.tile([P, P], F32, tag="diag")
                nc.scalar.activation(diag, ident, Copy, scale=a3[:, 0:1])

                ps_out = psum_pool.tile([P, D], F32, tag="ps_out")
                for h in range(2):
                    sl = slice(512 * h, 512 * (h + 1))
                    nc.tensor.matmul(
                        ps_out[:, sl], diag, xt[:, sl], start=True, stop=False
                    )
                    nc.tensor.matmul(
                        ps_out[:, sl], ones_col, dsum_row[:, sl], start=False, stop=True
                    )

                ot = op.tile([P, D], F32, tag="ot")
                nc.scalar.copy(ot, ps_out)
                nc.sync.dma_start(out=out[rows, :], in_=ot)

    nc.compile()
    return nc


def _get_program() -> Bass:
    key = f"{U2_ENGINE}-{REPEAT}"
    if key not in _CACHE:
        _CACHE[key] = _build()
    return _CACHE[key]


def _make_aux(weights: np.ndarray, bias: np.ndarray) -> np.ndarray:
    w = np.asarray(weights, dtype=np.float32)
    b = np.asarray(bias, dtype=np.float32)
    aux = np.zeros((5, D), dtype=np.float32)
    aux[0:3] = w
    aux[3] = b.sum(axis=0)
    aux[4, 0] = float(b[0] @ w[1])
    aux[4, 1] = float((b[0] + b[1]) @ w[2])
    return aux


def kernel(x: np.ndarray, weights: np.ndarray, bias: np.ndarray) -> np.ndarray:
    x = np.ascontiguousarray(np.asarray(x, dtype=np.float32))
    aux = _make_aux(weights, bias)
    nc = _get_program()
    in_maps = [
        {"x": x[i * RPC : (i + 1) * RPC], "aux": aux} for i in range(N_CORES)
    ]
    res = run_bass_kernel_spmd(nc, in_maps, list(range(N_CORES)))
    return np.concatenate([r["out"] for r in res.results], axis=0)
